# revision 22
# baseline (speedup 1.0000x reference)
"""GCN encoder (3-layer GCNConv + LayerNorm + ReLU + residual) on 8 TRN2
NeuronCores via Bass/Tile.

Sharding: nodes are partitioned across cores (graph parallel). Per layer each
core computes its own dinv-scaled xw shard (bf16), AllGathers the full table
to Shared DRAM, then aggregates its in-edges with batched `dma_gather` row
gathers (int16 indices, phase A/B around row 32768) and a PE matmul against
host-built one-hot S blocks ([128 slots, 128 dests] bf16 carrying the edge
weight), accumulating each dest window in PSUM.

Structure:
  - self-loop edges are not gathered; added via an identity matmul against
    the SBUF-resident own xwb window.
  - sources deduplicated within each (window, phase) group.
  - gathers grouped across GW windows per call (one A + one B call per
    group) to amortize the ~2us per-call fixed cost.
  - next layer's xw table build is interleaved into the aggregation loop so
    only the AllGather sits on the layer boundary.
"""

import numpy as np
import ml_dtypes

import concourse.bacc as bacc
import concourse.bass as bass
import concourse.mybir as mybir
from concourse.tile import TileContext
from concourse.bass_utils import run_bass_kernel_spmd
from concourse.library_config import mlp as mlp_library

F32 = mybir.dt.float32
BF16 = mybir.dt.bfloat16
I16 = mybir.dt.int16
AX = mybir.AxisListType
ALU = mybir.AluOpType
ACTF = mybir.ActivationFunctionType

BF16NP = ml_dtypes.bfloat16

GW = 3   # windows per gather group
WH = 25  # first-half windows (table half 1); rest are half 2


def _balance(wins, sizes):
    ng = (len(wins) + GW - 1) // GW
    order = sorted(wins, key=lambda w: -sizes[w])
    groups = [[] for _ in range(ng)]
    tot = [0] * ng
    for w in order:
        cands = [i for i in range(ng) if len(groups[i]) < GW]
        i = min(cands, key=lambda i: tot[i])
        groups[i].append(w)
        tot[i] += sizes[w]
    return [g for g in groups if g]


def _groups(W, nblk=None):
    """Groups of <=GW windows, balanced by block count, with all first-half
    windows (w < WH) in the leading groups so the half-1 AllGather of the
    next layer can fire mid-layer."""
    if nblk is None:
        sizes = [1] * W
    else:
        sizes = [int(nblk[w].sum()) for w in range(W)]
    return (_balance(list(range(WH)), sizes) +
            _balance(list(range(WH, W)), sizes))


# ----------------------------------------------------------------------------
# Host-side structure packing
# ----------------------------------------------------------------------------

def build_structure(edge_index, edge_weight, N, C, W, HALF=32768):
    NPC = N // C
    NP = W * 128
    src = np.asarray(edge_index[0], dtype=np.int64)
    dst = np.asarray(edge_index[1], dtype=np.int64)

    loop = np.arange(N, dtype=np.int64)
    dst2 = np.concatenate([dst, loop])
    w2 = np.concatenate([np.asarray(edge_weight, dtype=np.float32),
                         np.ones(N, dtype=np.float32)])

    deg_all = np.bincount(dst2, minlength=N)  # in-degree incl self loop

    rank = np.empty(N, dtype=np.int64)
    for c in range(C):
        lo, hi = c * NPC, (c + 1) * NPC
        order = np.argsort(-deg_all[lo:hi], kind="stable")
        rank[lo + order] = np.arange(NPC)
    # half-split table layout: half1 = windows [0, WH) of every core packed
    # as [core, 3200]; half2 = windows [WH, W) as [core, NP-3200].
    NP1 = WH * 128
    core_of = np.arange(N) // NPC
    in_h2 = rank >= NP1
    node_rel = np.where(in_h2, core_of * (NP - NP1) + (rank - NP1),
                        core_of * NP1 + rank)

    owner = dst // NPC
    spos = node_rel[src]
    s_isB = in_h2[src].astype(np.int64)
    dloc = rank[dst]

    owner2 = dst2 // NPC
    dloc2 = rank[dst2]

    per_core = []
    cntAB = np.zeros((C, W, 2), dtype=np.int64)
    for c in range(C):
        sel = owner == c
        e_spos = spos[sel]
        e_dloc = dloc[sel]
        e_w = np.asarray(edge_weight, dtype=np.float32)[sel]
        win = e_dloc // 128
        isB = s_isB[sel]
        o = np.lexsort((e_spos, isB, win))
        e_spos, e_dloc, e_w, win, isB = (
            e_spos[o], e_dloc[o], e_w[o], win[o], isB[o])
        grp = win * 2 + isB
        n = len(e_spos)
        newslot = np.ones(n, dtype=bool)
        if n > 1:
            newslot[1:] = (grp[1:] != grp[:-1]) | (e_spos[1:] != e_spos[:-1])
        cnt = np.zeros((W, 2), dtype=np.int64)
        np.add.at(cnt, (win[newslot], isB[newslot]), 1)
        cntAB[c] = cnt
        sel2 = owner2 == c
        per_core.append(dict(spos=e_spos, dloc=e_dloc, w=e_w, win=win,
                             isB=isB, newslot=newslot, cnt=cnt,
                             dloc_deg=dloc2[sel2], w_deg=w2[sel2]))

    nblk = (np.ceil(cntAB / 128.0).astype(np.int64)).max(axis=0)  # [W, 2]

    KDEG = np.zeros(W, dtype=np.int64)
    deg_pad = np.zeros((C, NP), dtype=np.int64)
    for c in range(C):
        lo = c * NPC
        deg_pad[c, rank[lo:lo + NPC]] = deg_all[lo:lo + NPC]
    for w in range(W):
        KDEG[w] = deg_pad[:, w * 128:(w + 1) * 128].max()

    # group-major arena layout: per group: [A w0 | A w1 | ... | B w0 | B w1...]
    # base[w, ph] = padded slot base of window w's phase-ph run.
    base = np.zeros((W, 2), dtype=np.int64)
    acc = 0
    for ws in _groups(W, nblk):
        for w in ws:
            base[w, 0] = acc
            acc += int(nblk[w, 0]) * 128
        for w in ws:
            base[w, 1] = acc
            acc += int(nblk[w, 1]) * 128
    TOT = acc
    assert TOT == int(nblk.sum()) * 128

    return dict(NPC=NPC, NP=NP, NT=NP * C, HALF=HALF, C=C, W=W,
                nblk=nblk, KDEG=KDEG, per_core=per_core, rank=rank,
                base=base, TOT=TOT, NP1=NP1)


def pack_core(st, c):
    """Build idx_img (int16), s_img (bf16) and wdeg_img (f32) for core c."""
    W, HALF = st["W"], st["HALF"]
    d = st["per_core"][c]
    spos, dloc, wv, win, isB = d["spos"], d["dloc"], d["w"], d["win"], d["isB"]
    newslot = d["newslot"]
    base = st["base"]
    TOT = st["TOT"]

    slot_cum = np.cumsum(newslot) - 1
    grp = win * 2 + isB
    first_of_grp = np.zeros(W * 2, dtype=np.int64)
    gstart = np.ones(len(grp), dtype=bool)
    if len(grp) > 1:
        gstart[1:] = grp[1:] != grp[:-1]
    first_of_grp[grp[gstart]] = slot_cum[gstart]
    slot_in_grp = slot_cum - first_of_grp[grp]
    pos = base[win, isB] + slot_in_grp

    idx_flat = np.zeros(TOT, dtype=np.int16)
    sl = newslot
    rel = spos[sl]
    assert rel.min() >= 0 and rel.max() < 32768
    idx_flat[pos[sl]] = rel.astype(np.int16)

    idx_img = np.tile(idx_flat.reshape(TOT // 16, 16).T, (8, 1))
    idx_img = np.ascontiguousarray(idx_img, dtype=np.int16)

    s_flat = np.zeros((128, TOT), dtype=np.float32)
    np.add.at(s_flat, (pos % 128, (pos // 128) * 128 + (dloc % 128)), wv)
    s_img = s_flat.astype(BF16NP)

    # wdeg image (loop-inclusive; deg reduce only)
    KDEG = st["KDEG"]
    NP = st["NP"]
    cols = []
    dl_all = d["dloc_deg"]
    w_all = d["w_deg"]
    order = np.argsort(dl_all, kind="stable")
    dl_s = dl_all[order]
    w_s = w_all[order]
    starts = np.searchsorted(dl_s, np.arange(NP))
    ends = np.searchsorted(dl_s, np.arange(NP) + 1)
    for w in range(W):
        K = int(KDEG[w])
        if K == 0:
            continue
        blk = np.zeros((128, K), dtype=np.float32)
        for p in range(128):
            dd = w * 128 + p
            s, e = starts[dd], ends[dd]
            if e == s:
                blk[p, 0] = 1.0  # pad dest: finite dinv (NaN poisons PE)
            else:
                blk[p, : e - s] = w_s[s:e]
        cols.append(blk)
    wdeg_img = np.concatenate(cols, axis=1)
    return idx_img, s_img, wdeg_img


# ----------------------------------------------------------------------------
# Bass program
# ----------------------------------------------------------------------------

def build_program(st, L, D=128):
    W = st["W"]
    NP = st["NP"]
    NT = st["NT"]
    HALF = st["HALF"]
    C = st["C"]
    nblk = st["nblk"]
    KDEG = st["KDEG"]
    TOT = st["TOT"]
    IDXW = TOT // 16
    KCOLS = int(KDEG.sum())
    groups = _groups(W, nblk)

    nc = bacc.Bacc("TRN2", target_bir_lowering=False, debug=True,
                   num_swdge_queues=4)

    x_in = nc.dram_tensor("x_shard", [NP, D], F32, kind="ExternalInput")
    idx_in = nc.dram_tensor("idx_img", [128, IDXW], I16, kind="ExternalInput")
    s_in = nc.dram_tensor("s_img", [128, TOT], BF16, kind="ExternalInput")
    wdeg_in = nc.dram_tensor("wdeg_img", [128, KCOLS], F32, kind="ExternalInput")
    wst_in = nc.dram_tensor("wst", [L, D, D], F32, kind="ExternalInput")
    bias_in = nc.dram_tensor("bias_b", [L, D, D], F32, kind="ExternalInput")
    gam_in = nc.dram_tensor("gamma_b", [L, D, D], F32, kind="ExternalInput")
    bet_in = nc.dram_tensor("beta_b", [L, D, D], F32, kind="ExternalInput")
    id_in = nc.dram_tensor("ident", [D, D], F32, kind="ExternalInput")
    idb_in = nc.dram_tensor("ident_bf", [D, D], BF16, kind="ExternalInput")
    out_t = nc.dram_tensor("out_shard", [NP, D], F32, kind="ExternalOutput")

    with TileContext(nc) as tc:
        with (
            tc.tile_pool(name="persist", bufs=1) as pp,
            tc.tile_pool(name="gath", bufs=2) as gp,
            tc.tile_pool(name="smat", bufs=2) as sp,
            tc.tile_pool(name="work", bufs=3) as wk,
            tc.tile_pool(name="tiny", bufs=4) as tn,
            tc.tile_pool(name="psum", bufs=2, space="PSUM") as ps,
            tc.tile_pool(name="psagg", bufs=4, space="PSUM") as pagg,
            tc.tile_pool(name="dram", bufs=1, space="DRAM") as dr,
        ):
            # ---- persistent SBUF state ----
            h = pp.tile([128, W, D], F32, tag="h")
            idx = pp.tile([128, IDXW], I16, tag="idx")
            wdeg = pp.tile([128, KCOLS], F32, tag="wdeg")
            wst = pp.tile([128, L * D], F32, tag="wst")
            biasb = pp.tile([128, L * D], F32, tag="biasb")
            gamb = pp.tile([128, L * D], F32, tag="gamb")
            betb = pp.tile([128, L * D], F32, tag="betb")
            ident = pp.tile([128, D], F32, tag="ident")
            identb = pp.tile([128, D], BF16, tag="identb")
            dinv = pp.tile([128, W], F32, tag="dinv")
            xwp = [pp.tile([128, W, D], BF16, name=f"xwp{i}", tag=f"xwp{i}")
                   for i in range(2)]

            nc.sync.dma_start(out=h[:, :, :],
                              in_=x_in[:].rearrange("(w p) f -> p w f", p=128))
            nc.sync.dma_start(out=idx[:, :], in_=idx_in[:, :])
            nc.sync.dma_start(out=wdeg[:, :], in_=wdeg_in[:, :])
            for l in range(L):
                for dst_t, src_t in ((wst, wst_in), (biasb, bias_in),
                                     (gamb, gam_in), (betb, bet_in)):
                    nc.sync.dma_start(out=dst_t[:, l * D:(l + 1) * D],
                                      in_=src_t[l, :, :])
            nc.sync.dma_start(out=ident[:, :], in_=id_in[:, :])
            nc.sync.dma_start(out=identb[:, :], in_=idb_in[:, :])

            nc.gpsimd.load_library(mlp_library)

            # ---- degree -> dinv ----
            deg = tn.tile([128, W], F32, tag="deg")
            off = 0
            for w in range(W):
                K = int(KDEG[w])
                nc.vector.tensor_reduce(deg[:, w:w + 1], wdeg[:, off:off + K],
                                        AX.X, ALU.add)
                off += K
            rdeg = tn.tile([128, W], F32, tag="rdeg")
            nc.vector.reciprocal(rdeg[:, :], deg[:, :])
            nc.scalar.sqrt(dinv[:, :], rdeg[:, :])

            NP1 = st["NP1"]
            T1, T2 = NP1 * C, (NP - NP1) * C
            tab1s = [dr.tile([T1, D], BF16, name=f"tab1_{i}", tag=f"tab1_{i}",
                             addr_space="Shared") for i in range(L)]
            tab2s = [dr.tile([T2, D], BF16, name=f"tab2_{i}", tag=f"tab2_{i}",
                             addr_space="Shared") for i in range(L)]
            xw_own = [dr.tile([NP, D], BF16, name=f"xwown{i}", tag=f"xwown{i}")
                      for i in range(2)]

            def ag_half(li, half):
                own = xw_own[li % 2]
                if half == 0:
                    nc.gpsimd.collective_compute(
                        "AllGather", ALU.bypass,
                        replica_groups=[list(range(C))],
                        ins=[own[0:NP1, :].opt()],
                        outs=[tab1s[li][:].opt()])
                else:
                    nc.gpsimd.collective_compute(
                        "AllGather", ALU.bypass,
                        replica_groups=[list(range(C))],
                        ins=[own[NP1:NP, :].opt()],
                        outs=[tab2s[li][:].opt()])

            def build_xw(li, w):
                """xwp[li%2][:, w, :] = bf16(dinv * (h_w @ Ws_li^T)); DMA to own."""
                wst_l = wst[:, li * D:(li + 1) * D]
                own = xw_own[li % 2]
                xw = xwp[li % 2]
                hT = ps.tile([128, D], F32, tag="hT")
                nc.tensor.transpose(hT[:, :], h[:, w, :], ident[:, :])
                hTs = wk.tile([128, D], F32, tag="hTs")
                nc.scalar.activation(hTs[:, :], hT[:, :], ACTF.Copy)
                mm = ps.tile([128, D], F32, tag="mm")
                nc.tensor.matmul(mm[:, :], hTs[:, :], wst_l)
                nc.scalar.activation(xw[:, w, :], mm[:, :], ACTF.Copy,
                                     scale=dinv[:, w:w + 1])
                nc.sync.dma_start(out=own[w * 128:(w + 1) * 128, :],
                                  in_=xw[:, w, :])

            # prologue: layer-0 table (half-1 AG as soon as its windows done)
            for w in range(W):
                build_xw(0, w)
                if w == WH - 1:
                    ag_half(0, 0)
            ag_half(0, 1)

            qn = 0
            for li in range(L):
                tab1, tab2 = tab1s[li], tab2s[li]
                xw = xwp[li % 2]
                built = 0
                for ws in groups:
                    nAs = [int(nblk[w, 0]) for w in ws]
                    nBs = [int(nblk[w, 1]) for w in ws]
                    gA, gB = sum(nAs), sum(nBs)
                    gT = gA + gB
                    # arena base (blocks) of this group
                    gbase = int(st["base"][ws[0], 0]) // 128
                    g = gp.tile([128, gT, D], BF16, tag="g")
                    if gA:
                        nc.gpsimd.dma_gather(
                            g[:, 0:gA, :], tab1[:, :],
                            idx[:, gbase * 8:(gbase + gA) * 8],
                            gA * 128, gA * 128, D, single_packet=False,
                            queue_num=qn % 4)
                        qn += 1
                    if gB:
                        nc.gpsimd.dma_gather(
                            g[:, gA:gT, :], tab2[:, :],
                            idx[:, (gbase + gA) * 8:(gbase + gT) * 8],
                            gB * 128, gB * 128, D, single_packet=False,
                            queue_num=qn % 4)
                        qn += 1
                    s_t = sp.tile([128, gT, 128], BF16, tag="s_t")
                    nc.sync.dma_start(
                        out=s_t[:, :, :],
                        in_=s_in[:, gbase * 128:(gbase + gT) * 128])
                    # per-window aggregation
                    offA = 0
                    offB = gA
                    for wi, w in enumerate(ws):
                        blocks = (list(range(offA, offA + nAs[wi])) +
                                  list(range(offB, offB + nBs[wi])))
                        nb = len(blocks)
                        agg = pagg.tile([128, D], F32, tag="agg")
                        nc.tensor.matmul(agg[:, :], identb[:, :], xw[:, w, :],
                                         start=True, stop=False)
                        for k, b in enumerate(blocks):
                            nc.tensor.matmul(agg[:, :], s_t[:, b, :],
                                             g[:, b, :], start=False,
                                             stop=(k == nb - 1))
                        offA += nAs[wi]
                        offB += nBs[wi]
                        x0 = wk.tile([128, D], F32, tag="x0")
                        nc.scalar.activation(x0[:, :], agg[:, :], ACTF.Copy,
                                             scale=dinv[:, w:w + 1])
                        nc.vector.tensor_add(x0[:, :], x0[:, :],
                                             biasb[:, li * D:(li + 1) * D])
                        sx = tn.tile([128, 1], F32, tag="sx")
                        nc.vector.tensor_reduce(sx[:, :], x0[:, :], AX.X,
                                                ALU.add)
                        sq = tn.tile([128, 1], F32, tag="sq")
                        sqs = wk.tile([128, D], F32, tag="sqs")
                        nc.scalar.activation(sqs[:, :], x0[:, :], ACTF.Square,
                                             accum_out=sq[:, :])
                        negmu = tn.tile([128, 1], F32, tag="negmu")
                        nc.vector.tensor_scalar_mul(negmu[:, :], sx[:, :],
                                                    -1.0 / D)
                        ms = tn.tile([128, 1], F32, tag="ms")
                        nc.vector.tensor_scalar(ms[:, :], sq[:, :], 1.0 / D,
                                                1e-5, ALU.mult, ALU.add)
                        mu2 = tn.tile([128, 1], F32, tag="mu2")
                        nc.vector.tensor_mul(mu2[:, :], negmu[:, :],
                                             negmu[:, :])
                        var = tn.tile([128, 1], F32, tag="var")
                        nc.vector.tensor_sub(var[:, :], ms[:, :], mu2[:, :])
                        rv = tn.tile([128, 1], F32, tag="rv")
                        nc.vector.reciprocal(rv[:, :], var[:, :])
                        rstd = tn.tile([128, 1], F32, tag="rstd")
                        nc.scalar.sqrt(rstd[:, :], rv[:, :])
                        nnmr = tn.tile([128, 1], F32, tag="nnmr")
                        nc.vector.tensor_mul(nnmr[:, :], negmu[:, :],
                                             rstd[:, :])
                        t = wk.tile([128, D], F32, tag="t")
                        nc.scalar.activation(t[:, :], x0[:, :], ACTF.Identity,
                                             scale=rstd[:, :], bias=nnmr[:, :])
                        nc.vector.tensor_mul(t[:, :], t[:, :],
                                             gamb[:, li * D:(li + 1) * D])
                        nc.vector.tensor_add(t[:, :], t[:, :],
                                             betb[:, li * D:(li + 1) * D])
                        if li < L - 1:
                            nc.scalar.activation(t[:, :], t[:, :], ACTF.Relu)
                        nc.vector.tensor_add(h[:, w, :], t[:, :], h[:, w, :])
                        if li + 1 < L:
                            build_xw(li + 1, w)
                            built += 1
                            if built == WH:
                                ag_half(li + 1, 0)
                        else:
                            nc.sync.dma_start(
                                out=out_t[w * 128:(w + 1) * 128, :],
                                in_=h[:, w, :])
                if li + 1 < L:
                    ag_half(li + 1, 1)

    nc.compile()
    return nc


# ----------------------------------------------------------------------------
# Full kernel entry
# ----------------------------------------------------------------------------

def _kernel_impl(x, edge_index, edge_weight, Ws, bs, gammas, betas,
                 C=8, W=49, HALF=32768, trace=False):
    N, D = x.shape
    L = Ws.shape[0]
    st = build_structure(edge_index, edge_weight, N, C, W, HALF)
    NP, NPC = st["NP"], st["NPC"]

    ident = np.eye(D, dtype=np.float32)
    ident_bf = np.eye(D, dtype=np.float32).astype(BF16NP)
    wst = np.ascontiguousarray(np.transpose(np.asarray(Ws), (0, 2, 1))).astype(np.float32)
    bias_b = np.ascontiguousarray(
        np.broadcast_to(np.asarray(bs)[:, None, :], (L, D, D))).astype(np.float32)
    gam_b = np.ascontiguousarray(
        np.broadcast_to(np.asarray(gammas)[:, None, :], (L, D, D))).astype(np.float32)
    bet_b = np.ascontiguousarray(
        np.broadcast_to(np.asarray(betas)[:, None, :], (L, D, D))).astype(np.float32)

    in_maps = []
    for c in range(C):
        idx_img, s_img, wdeg_img = pack_core(st, c)
        xs = np.zeros((NP, D), dtype=np.float32)
        lo = c * NPC
        xs[st["rank"][lo:lo + NPC]] = np.asarray(x[lo:lo + NPC], dtype=np.float32)
        in_maps.append(dict(x_shard=xs, idx_img=idx_img, s_img=s_img,
                            wdeg_img=wdeg_img, wst=wst, bias_b=bias_b,
                            gamma_b=gam_b, beta_b=bet_b, ident=ident,
                            ident_bf=ident_bf))

    nc = build_program(st, L, D)
    res = run_bass_kernel_spmd(nc, in_maps, list(range(C)), trace=trace)

    out = np.empty((N, D), dtype=np.float32)
    for c in range(C):
        lo = c * NPC
        sh = res.results[c]["out_shard"]
        out[lo:lo + NPC] = sh[st["rank"][lo:lo + NPC]]
    return out, res


def kernel(x, edge_index, edge_weight, Ws, bs, gammas, betas):
    out, _ = _kernel_impl(np.asarray(x), np.asarray(edge_index),
                          np.asarray(edge_weight), np.asarray(Ws),
                          np.asarray(bs), np.asarray(gammas), np.asarray(betas))
    return out


# revision 23
# speedup vs baseline: 1.2252x; 1.2252x over previous
"""GCN encoder (3-layer GCNConv + LayerNorm + ReLU + residual) on 8 TRN2
NeuronCores via Bass/Tile.

Sharding: nodes are partitioned across cores (graph parallel). Per layer each
core computes its own dinv-scaled xw shard (bf16), AllGathers the full table
to Shared DRAM, then aggregates its in-edges with batched `dma_gather` row
gathers (int16 indices, phase A/B around row 32768) and a PE matmul against
host-built one-hot S blocks ([128 slots, 128 dests] bf16 carrying the edge
weight), accumulating each dest window in PSUM.

Structure:
  - self-loop edges are not gathered; added via an identity matmul against
    the SBUF-resident own xwb window.
  - sources deduplicated within each (window, phase) group.
  - gathers grouped across GW windows per call (one A + one B call per
    group) to amortize the ~2us per-call fixed cost.
  - next layer's xw table build is interleaved into the aggregation loop so
    only the AllGather sits on the layer boundary.
"""

import numpy as np
import ml_dtypes

import concourse.bacc as bacc
import concourse.bass as bass
import concourse.mybir as mybir
from concourse.tile import TileContext
from concourse.bass_utils import run_bass_kernel_spmd
from concourse.library_config import mlp as mlp_library

F32 = mybir.dt.float32
BF16 = mybir.dt.bfloat16
I16 = mybir.dt.int16
AX = mybir.AxisListType
ALU = mybir.AluOpType
ACTF = mybir.ActivationFunctionType

BF16NP = ml_dtypes.bfloat16

GW = 2   # windows per gather group
WH = 25  # first-half windows (table half 1); rest are half 2


def _balance(wins, sizes):
    ng = (len(wins) + GW - 1) // GW
    order = sorted(wins, key=lambda w: -sizes[w])
    groups = [[] for _ in range(ng)]
    tot = [0] * ng
    for w in order:
        cands = [i for i in range(ng) if len(groups[i]) < GW]
        i = min(cands, key=lambda i: tot[i])
        groups[i].append(w)
        tot[i] += sizes[w]
    return [g for g in groups if g]


def _groups(W, nblk=None):
    """Groups of <=GW windows, balanced by block count, with all first-half
    windows (w < WH) in the leading groups so the half-1 AllGather of the
    next layer can fire mid-layer."""
    if nblk is None:
        sizes = [1] * W
    else:
        sizes = [int(nblk[w].sum()) for w in range(W)]
    return (_balance(list(range(WH)), sizes) +
            _balance(list(range(WH, W)), sizes))


# ----------------------------------------------------------------------------
# Host-side structure packing
# ----------------------------------------------------------------------------

def build_structure(edge_index, edge_weight, N, C, W, HALF=32768):
    NPC = N // C
    NP = W * 128
    src = np.asarray(edge_index[0], dtype=np.int64)
    dst = np.asarray(edge_index[1], dtype=np.int64)

    loop = np.arange(N, dtype=np.int64)
    dst2 = np.concatenate([dst, loop])
    w2 = np.concatenate([np.asarray(edge_weight, dtype=np.float32),
                         np.ones(N, dtype=np.float32)])

    deg_all = np.bincount(dst2, minlength=N)  # in-degree incl self loop

    rank = np.empty(N, dtype=np.int64)
    for c in range(C):
        lo, hi = c * NPC, (c + 1) * NPC
        order = np.argsort(-deg_all[lo:hi], kind="stable")
        rank[lo + order] = np.arange(NPC)
    # half-split table layout: half1 = windows [0, WH) of every core packed
    # as [core, 3200]; half2 = windows [WH, W) as [core, NP-3200].
    NP1 = WH * 128
    core_of = np.arange(N) // NPC
    in_h2 = rank >= NP1
    node_rel = np.where(in_h2, core_of * (NP - NP1) + (rank - NP1),
                        core_of * NP1 + rank)

    owner = dst // NPC
    spos = node_rel[src]
    s_isB = in_h2[src].astype(np.int64)
    dloc = rank[dst]

    owner2 = dst2 // NPC
    dloc2 = rank[dst2]

    per_core = []
    cntAB = np.zeros((C, W, 2), dtype=np.int64)
    for c in range(C):
        sel = owner == c
        e_spos = spos[sel]
        e_dloc = dloc[sel]
        e_w = np.asarray(edge_weight, dtype=np.float32)[sel]
        win = e_dloc // 128
        isB = s_isB[sel]
        o = np.lexsort((e_spos, isB, win))
        e_spos, e_dloc, e_w, win, isB = (
            e_spos[o], e_dloc[o], e_w[o], win[o], isB[o])
        grp = win * 2 + isB
        n = len(e_spos)
        newslot = np.ones(n, dtype=bool)
        if n > 1:
            newslot[1:] = (grp[1:] != grp[:-1]) | (e_spos[1:] != e_spos[:-1])
        cnt = np.zeros((W, 2), dtype=np.int64)
        np.add.at(cnt, (win[newslot], isB[newslot]), 1)
        cntAB[c] = cnt
        sel2 = owner2 == c
        per_core.append(dict(spos=e_spos, dloc=e_dloc, w=e_w, win=win,
                             isB=isB, newslot=newslot, cnt=cnt,
                             dloc_deg=dloc2[sel2], w_deg=w2[sel2]))

    nblk = (np.ceil(cntAB / 128.0).astype(np.int64)).max(axis=0)  # [W, 2]

    KDEG = np.zeros(W, dtype=np.int64)
    deg_pad = np.zeros((C, NP), dtype=np.int64)
    for c in range(C):
        lo = c * NPC
        deg_pad[c, rank[lo:lo + NPC]] = deg_all[lo:lo + NPC]
    for w in range(W):
        KDEG[w] = deg_pad[:, w * 128:(w + 1) * 128].max()

    # group-major arena layout: per group: [A w0 | A w1 | ... | B w0 | B w1...]
    # base[w, ph] = padded slot base of window w's phase-ph run.
    base = np.zeros((W, 2), dtype=np.int64)
    acc = 0
    for ws in _groups(W, nblk):
        for w in ws:
            base[w, 0] = acc
            acc += int(nblk[w, 0]) * 128
        for w in ws:
            base[w, 1] = acc
            acc += int(nblk[w, 1]) * 128
    TOT = acc
    assert TOT == int(nblk.sum()) * 128

    return dict(NPC=NPC, NP=NP, NT=NP * C, HALF=HALF, C=C, W=W,
                nblk=nblk, KDEG=KDEG, per_core=per_core, rank=rank,
                base=base, TOT=TOT, NP1=NP1)


def pack_core(st, c):
    """Build idx_img (int16), s_img (bf16) and wdeg_img (f32) for core c."""
    W, HALF = st["W"], st["HALF"]
    d = st["per_core"][c]
    spos, dloc, wv, win, isB = d["spos"], d["dloc"], d["w"], d["win"], d["isB"]
    newslot = d["newslot"]
    base = st["base"]
    TOT = st["TOT"]

    slot_cum = np.cumsum(newslot) - 1
    grp = win * 2 + isB
    first_of_grp = np.zeros(W * 2, dtype=np.int64)
    gstart = np.ones(len(grp), dtype=bool)
    if len(grp) > 1:
        gstart[1:] = grp[1:] != grp[:-1]
    first_of_grp[grp[gstart]] = slot_cum[gstart]
    slot_in_grp = slot_cum - first_of_grp[grp]
    pos = base[win, isB] + slot_in_grp

    idx_flat = np.zeros(TOT, dtype=np.int16)
    sl = newslot
    rel = spos[sl]
    assert rel.min() >= 0 and rel.max() < 32768
    idx_flat[pos[sl]] = rel.astype(np.int16)

    idx_img = np.tile(idx_flat.reshape(TOT // 16, 16).T, (8, 1))
    idx_img = np.ascontiguousarray(idx_img, dtype=np.int16)

    s_flat = np.zeros((128, TOT), dtype=np.float32)
    np.add.at(s_flat, (pos % 128, (pos // 128) * 128 + (dloc % 128)), wv)
    s_img = s_flat.astype(BF16NP)

    # wdeg image (loop-inclusive; deg reduce only)
    KDEG = st["KDEG"]
    NP = st["NP"]
    cols = []
    dl_all = d["dloc_deg"]
    w_all = d["w_deg"]
    order = np.argsort(dl_all, kind="stable")
    dl_s = dl_all[order]
    w_s = w_all[order]
    starts = np.searchsorted(dl_s, np.arange(NP))
    ends = np.searchsorted(dl_s, np.arange(NP) + 1)
    for w in range(W):
        K = int(KDEG[w])
        if K == 0:
            continue
        blk = np.zeros((128, K), dtype=np.float32)
        for p in range(128):
            dd = w * 128 + p
            s, e = starts[dd], ends[dd]
            if e == s:
                blk[p, 0] = 1.0  # pad dest: finite dinv (NaN poisons PE)
            else:
                blk[p, : e - s] = w_s[s:e]
        cols.append(blk)
    wdeg_img = np.concatenate(cols, axis=1)
    return idx_img, s_img, wdeg_img


# ----------------------------------------------------------------------------
# Bass program
# ----------------------------------------------------------------------------

def build_program(st, L, D=128):
    W = st["W"]
    NP = st["NP"]
    NT = st["NT"]
    HALF = st["HALF"]
    C = st["C"]
    nblk = st["nblk"]
    KDEG = st["KDEG"]
    TOT = st["TOT"]
    IDXW = TOT // 16
    KCOLS = int(KDEG.sum())
    groups = _groups(W, nblk)

    nc = bacc.Bacc("TRN2", target_bir_lowering=False, debug=True,
                   num_swdge_queues=4)

    x_in = nc.dram_tensor("x_shard", [NP, D], F32, kind="ExternalInput")
    idx_in = nc.dram_tensor("idx_img", [128, IDXW], I16, kind="ExternalInput")
    s_in = nc.dram_tensor("s_img", [128, TOT], BF16, kind="ExternalInput")
    wdeg_in = nc.dram_tensor("wdeg_img", [128, KCOLS], F32, kind="ExternalInput")
    wst_in = nc.dram_tensor("wst", [L, D, D], F32, kind="ExternalInput")
    bias_in = nc.dram_tensor("bias_b", [L, D, D], F32, kind="ExternalInput")
    gam_in = nc.dram_tensor("gamma_b", [L, D, D], F32, kind="ExternalInput")
    bet_in = nc.dram_tensor("beta_b", [L, D, D], F32, kind="ExternalInput")
    id_in = nc.dram_tensor("ident", [D, D], F32, kind="ExternalInput")
    idb_in = nc.dram_tensor("ident_bf", [D, D], BF16, kind="ExternalInput")
    out_t = nc.dram_tensor("out_shard", [NP, D], F32, kind="ExternalOutput")

    with TileContext(nc) as tc:
        with (
            tc.tile_pool(name="persist", bufs=1) as pp,
            tc.tile_pool(name="gath", bufs=3) as gp,
            tc.tile_pool(name="smat", bufs=2) as sp,
            tc.tile_pool(name="work", bufs=3) as wk,
            tc.tile_pool(name="tiny", bufs=4) as tn,
            tc.tile_pool(name="psum", bufs=2, space="PSUM") as ps,
            tc.tile_pool(name="psagg", bufs=4, space="PSUM") as pagg,
            tc.tile_pool(name="dram", bufs=1, space="DRAM") as dr,
        ):
            # ---- persistent SBUF state ----
            h = pp.tile([128, W, D], F32, tag="h")
            idx = pp.tile([128, IDXW], I16, tag="idx")
            wdeg = pp.tile([128, KCOLS], F32, tag="wdeg")
            wst = pp.tile([128, L * D], F32, tag="wst")
            biasb = pp.tile([128, L * D], F32, tag="biasb")
            gamb = pp.tile([128, L * D], F32, tag="gamb")
            betb = pp.tile([128, L * D], F32, tag="betb")
            ident = pp.tile([128, D], F32, tag="ident")
            identb = pp.tile([128, D], BF16, tag="identb")
            dinv = pp.tile([128, W], F32, tag="dinv")
            xwp = [pp.tile([128, W, D], BF16, name=f"xwp{i}", tag=f"xwp{i}")
                   for i in range(2)]

            nc.sync.dma_start(out=h[:, :, :],
                              in_=x_in[:].rearrange("(w p) f -> p w f", p=128))
            nc.sync.dma_start(out=idx[:, :], in_=idx_in[:, :])
            nc.sync.dma_start(out=wdeg[:, :], in_=wdeg_in[:, :])
            for l in range(L):
                for dst_t, src_t in ((wst, wst_in), (biasb, bias_in),
                                     (gamb, gam_in), (betb, bet_in)):
                    nc.sync.dma_start(out=dst_t[:, l * D:(l + 1) * D],
                                      in_=src_t[l, :, :])
            nc.sync.dma_start(out=ident[:, :], in_=id_in[:, :])
            nc.sync.dma_start(out=identb[:, :], in_=idb_in[:, :])

            nc.gpsimd.load_library(mlp_library)

            # ---- degree -> dinv ----
            deg = tn.tile([128, W], F32, tag="deg")
            off = 0
            for w in range(W):
                K = int(KDEG[w])
                nc.vector.tensor_reduce(deg[:, w:w + 1], wdeg[:, off:off + K],
                                        AX.X, ALU.add)
                off += K
            rdeg = tn.tile([128, W], F32, tag="rdeg")
            nc.vector.reciprocal(rdeg[:, :], deg[:, :])
            nc.scalar.sqrt(dinv[:, :], rdeg[:, :])

            NP1 = st["NP1"]
            T1, T2 = NP1 * C, (NP - NP1) * C
            tab1s = [dr.tile([T1, D], BF16, name=f"tab1_{i}", tag=f"tab1_{i}",
                             addr_space="Shared") for i in range(L)]
            tab2s = [dr.tile([T2, D], BF16, name=f"tab2_{i}", tag=f"tab2_{i}",
                             addr_space="Shared") for i in range(L)]
            xw_own = [dr.tile([NP, D], BF16, name=f"xwown{i}", tag=f"xwown{i}")
                      for i in range(2)]

            def ag_half(li, half):
                own = xw_own[li % 2]
                if half == 0:
                    nc.gpsimd.collective_compute(
                        "AllGather", ALU.bypass,
                        replica_groups=[list(range(C))],
                        ins=[own[0:NP1, :].opt()],
                        outs=[tab1s[li][:].opt()])
                else:
                    nc.gpsimd.collective_compute(
                        "AllGather", ALU.bypass,
                        replica_groups=[list(range(C))],
                        ins=[own[NP1:NP, :].opt()],
                        outs=[tab2s[li][:].opt()])

            def build_xw(li, w):
                """xwp[li%2][:, w, :] = bf16(dinv * (h_w @ Ws_li^T)); DMA to own."""
                wst_l = wst[:, li * D:(li + 1) * D]
                own = xw_own[li % 2]
                xw = xwp[li % 2]
                hT = ps.tile([128, D], F32, tag="hT")
                nc.tensor.transpose(hT[:, :], h[:, w, :], ident[:, :])
                hTs = wk.tile([128, D], F32, tag="hTs")
                nc.scalar.activation(hTs[:, :], hT[:, :], ACTF.Copy)
                mm = ps.tile([128, D], F32, tag="mm")
                nc.tensor.matmul(mm[:, :], hTs[:, :], wst_l)
                nc.scalar.activation(xw[:, w, :], mm[:, :], ACTF.Copy,
                                     scale=dinv[:, w:w + 1])
                nc.sync.dma_start(out=own[w * 128:(w + 1) * 128, :],
                                  in_=xw[:, w, :])

            # prologue: layer-0 table (half-1 AG as soon as its windows done)
            for w in range(W):
                build_xw(0, w)
                if w == WH - 1:
                    ag_half(0, 0)
            ag_half(0, 1)

            qn = 0
            for li in range(L):
                tab1, tab2 = tab1s[li], tab2s[li]
                xw = xwp[li % 2]
                built = 0
                for ws in groups:
                    nAs = [int(nblk[w, 0]) for w in ws]
                    nBs = [int(nblk[w, 1]) for w in ws]
                    gA, gB = sum(nAs), sum(nBs)
                    gT = gA + gB
                    # arena base (blocks) of this group
                    gbase = int(st["base"][ws[0], 0]) // 128
                    g = gp.tile([128, gT, D], BF16, tag="g")
                    if gA:
                        nc.gpsimd.dma_gather(
                            g[:, 0:gA, :], tab1[:, :],
                            idx[:, gbase * 8:(gbase + gA) * 8],
                            gA * 128, gA * 128, D, single_packet=False,
                            queue_num=qn % 4)
                        qn += 1
                    if gB:
                        nc.gpsimd.dma_gather(
                            g[:, gA:gT, :], tab2[:, :],
                            idx[:, (gbase + gA) * 8:(gbase + gT) * 8],
                            gB * 128, gB * 128, D, single_packet=False,
                            queue_num=qn % 4)
                        qn += 1
                    s_t = sp.tile([128, gT, 128], BF16, tag="s_t")
                    nc.sync.dma_start(
                        out=s_t[:, :, :],
                        in_=s_in[:, gbase * 128:(gbase + gT) * 128])
                    # per-window aggregation
                    offA = 0
                    offB = gA
                    for wi, w in enumerate(ws):
                        blocks = (list(range(offA, offA + nAs[wi])) +
                                  list(range(offB, offB + nBs[wi])))
                        nb = len(blocks)
                        agg = pagg.tile([128, D], F32, tag="agg")
                        nc.tensor.matmul(agg[:, :], identb[:, :], xw[:, w, :],
                                         start=True, stop=False)
                        for k, b in enumerate(blocks):
                            nc.tensor.matmul(agg[:, :], s_t[:, b, :],
                                             g[:, b, :], start=False,
                                             stop=(k == nb - 1))
                        offA += nAs[wi]
                        offB += nBs[wi]
                        x0 = wk.tile([128, D], F32, tag="x0")
                        nc.scalar.activation(x0[:, :], agg[:, :], ACTF.Copy,
                                             scale=dinv[:, w:w + 1])
                        nc.vector.tensor_add(x0[:, :], x0[:, :],
                                             biasb[:, li * D:(li + 1) * D])
                        sx = tn.tile([128, 1], F32, tag="sx")
                        nc.vector.tensor_reduce(sx[:, :], x0[:, :], AX.X,
                                                ALU.add)
                        sq = tn.tile([128, 1], F32, tag="sq")
                        sqs = wk.tile([128, D], F32, tag="sqs")
                        nc.scalar.activation(sqs[:, :], x0[:, :], ACTF.Square,
                                             accum_out=sq[:, :])
                        negmu = tn.tile([128, 1], F32, tag="negmu")
                        nc.vector.tensor_scalar_mul(negmu[:, :], sx[:, :],
                                                    -1.0 / D)
                        ms = tn.tile([128, 1], F32, tag="ms")
                        nc.vector.tensor_scalar(ms[:, :], sq[:, :], 1.0 / D,
                                                1e-5, ALU.mult, ALU.add)
                        mu2 = tn.tile([128, 1], F32, tag="mu2")
                        nc.vector.tensor_mul(mu2[:, :], negmu[:, :],
                                             negmu[:, :])
                        var = tn.tile([128, 1], F32, tag="var")
                        nc.vector.tensor_sub(var[:, :], ms[:, :], mu2[:, :])
                        rv = tn.tile([128, 1], F32, tag="rv")
                        nc.vector.reciprocal(rv[:, :], var[:, :])
                        rstd = tn.tile([128, 1], F32, tag="rstd")
                        nc.scalar.sqrt(rstd[:, :], rv[:, :])
                        nnmr = tn.tile([128, 1], F32, tag="nnmr")
                        nc.vector.tensor_mul(nnmr[:, :], negmu[:, :],
                                             rstd[:, :])
                        t = wk.tile([128, D], F32, tag="t")
                        nc.scalar.activation(t[:, :], x0[:, :], ACTF.Identity,
                                             scale=rstd[:, :], bias=nnmr[:, :])
                        nc.vector.tensor_mul(t[:, :], t[:, :],
                                             gamb[:, li * D:(li + 1) * D])
                        nc.vector.tensor_add(t[:, :], t[:, :],
                                             betb[:, li * D:(li + 1) * D])
                        if li < L - 1:
                            nc.scalar.activation(t[:, :], t[:, :], ACTF.Relu)
                        nc.vector.tensor_add(h[:, w, :], t[:, :], h[:, w, :])
                        if li + 1 < L:
                            build_xw(li + 1, w)
                            built += 1
                            if built == WH:
                                ag_half(li + 1, 0)
                        else:
                            nc.sync.dma_start(
                                out=out_t[w * 128:(w + 1) * 128, :],
                                in_=h[:, w, :])
                if li + 1 < L:
                    ag_half(li + 1, 1)

    nc.compile()
    return nc


# ----------------------------------------------------------------------------
# Full kernel entry
# ----------------------------------------------------------------------------

def _kernel_impl(x, edge_index, edge_weight, Ws, bs, gammas, betas,
                 C=8, W=49, HALF=32768, trace=False):
    N, D = x.shape
    L = Ws.shape[0]
    st = build_structure(edge_index, edge_weight, N, C, W, HALF)
    NP, NPC = st["NP"], st["NPC"]

    ident = np.eye(D, dtype=np.float32)
    ident_bf = np.eye(D, dtype=np.float32).astype(BF16NP)
    wst = np.ascontiguousarray(np.transpose(np.asarray(Ws), (0, 2, 1))).astype(np.float32)
    bias_b = np.ascontiguousarray(
        np.broadcast_to(np.asarray(bs)[:, None, :], (L, D, D))).astype(np.float32)
    gam_b = np.ascontiguousarray(
        np.broadcast_to(np.asarray(gammas)[:, None, :], (L, D, D))).astype(np.float32)
    bet_b = np.ascontiguousarray(
        np.broadcast_to(np.asarray(betas)[:, None, :], (L, D, D))).astype(np.float32)

    in_maps = []
    for c in range(C):
        idx_img, s_img, wdeg_img = pack_core(st, c)
        xs = np.zeros((NP, D), dtype=np.float32)
        lo = c * NPC
        xs[st["rank"][lo:lo + NPC]] = np.asarray(x[lo:lo + NPC], dtype=np.float32)
        in_maps.append(dict(x_shard=xs, idx_img=idx_img, s_img=s_img,
                            wdeg_img=wdeg_img, wst=wst, bias_b=bias_b,
                            gamma_b=gam_b, beta_b=bet_b, ident=ident,
                            ident_bf=ident_bf))

    nc = build_program(st, L, D)
    res = run_bass_kernel_spmd(nc, in_maps, list(range(C)), trace=trace)

    out = np.empty((N, D), dtype=np.float32)
    for c in range(C):
        lo = c * NPC
        sh = res.results[c]["out_shard"]
        out[lo:lo + NPC] = sh[st["rank"][lo:lo + NPC]]
    return out, res


def kernel(x, edge_index, edge_weight, Ws, bs, gammas, betas):
    out, _ = _kernel_impl(np.asarray(x), np.asarray(edge_index),
                          np.asarray(edge_weight), np.asarray(Ws),
                          np.asarray(bs), np.asarray(gammas), np.asarray(betas))
    return out
